# revision 36
# baseline (speedup 1.0000x reference)
"""AFT-Full attention on 8 TRN2 NeuronCores (Bass/Tile, no collectives).

Reference math (B=2, TQ=TKV=512, DIM=512, HID=128, BDIM=128):
    qh  = q @ qW_w.T + qW_b
    k   = kv @ kW_w.T + kW_b
    v   = kv @ vW_w.T + vW_b
    wb  = w_bias_u @ w_bias_v                       # (TQ, TKV)
    A   = exp(k[:,None] + wb[None,:,:,None])        # (B,TQ,TKV,HID)
    out = sigmoid(qh) * (sum_s A*v / sum_s A)

Factorization: exp(k + wb) = exp(k) * exp(wb) collapses the giant A
intermediate into plain matmuls:
    num[t,h] = sum_s exp(wb[t,s]) * (exp(k0[s,h]) * v0[s,h])
    den[t,h] = sum_s exp(wb[t,s]) *  exp(k0[s,h])
The k-projection bias cancels exactly in num/den; the v bias is a pure
per-h additive term:  out = sigmoid(qh) * (num0/den0 + vW_b).

Sharding: the 1024 flattened (b, t) query rows split into 8 blocks of 128 —
core i handles batch b=i//4, queries t in [128*(i%4), 128*(i%4)+128).
Each core only needs kv[b], so no collectives (their ~7us latency floor
exceeds this whole kernel).

Host-side packing gives the device natural matmul layouts (contraction on
partitions, zero on-device transposes) AND DMA-friendly lines: each DRAM
slab is laid out exactly as its SBUF tile, so every partition reads one
contiguous 2-6.5KB run per slab (big descriptors -> near line-rate DMA).

    slab1: [kvWT(4x256) | wbv(512) | uT(128)]          (128, 1664)
    slab2: kv s-half 0, pre-tiled [sc, dc, 128]        (128, 1024)
    slab3: kv s-half 1, pre-tiled [sc, dc, 128]        (128, 1024)
    slab4: [qWT(4x128) | qT(4x128) | -qb | vb]         (128, 1026)

The kv halves are DMA'd as s-quarters so each projection chunk unblocks on
its own semaphore (finer straggler isolation on the shared SDMA engines).

dtype strategy: no on-device casts at all.  The f32 slabs are typed
float32r end-to-end; fp32r matmuls with a moving dim >= 256 run at full
bf16 rate reading f32 bits directly (the narrow wbias/qh matmuls pay
2-4x on 128 cols each, off the critical path).  The exp/sigmoid outputs
(ek, ekv, expwb) are written bf16 by ACT/DVE for the attention matmuls.
PSUM accumulation is f32 everywhere.  A run of dummy matmuls during the
DMA stream ramps the tensor engine clock (0.6->1.2->2.4GHz) so the real
matmuls issue at full rate.  sigmoid(qh) is computed as 1/(1+exp(-qh-qb))
so the ACT engine only ever loads the EXP table (saves 2x 1.28us table
switches).
"""

import numpy as np

import concourse.bass as bass
import concourse.mybir as mybir
import concourse.tile as tile
from concourse import bacc
from concourse.bass_utils import run_bass_kernel_spmd

B, TQ, TKV, DIM, HID, BDIM = 2, 512, 512, 512, 128, 128
N_CORES = 8
R = (B * TQ) // N_CORES  # 128 query rows per core
P = 128
DC = DIM // P  # 4 contraction chunks for d
SC = TKV // P  # 4 contraction chunks for s
F32 = mybir.dt.float32
BF16 = mybir.dt.bfloat16
ACT = mybir.ActivationFunctionType
N_WARMUP = 24
SLAB_ORDER = "1 Q0 Q1 Q2 Q3 4"
QH_EARLY = True
DUAL_RING = ""
KV_QUARTERS = True
OUT_SCALAR = False
S1_SWDGE = False
SPLIT_EPI = False
NO_MEMSET = True
MEMSET_ENG = "gps"
OUT_PRIMER = False
QW_IN_S1 = False
EPI_DIVIDE = False
INTER_FILLER = 2
KV_MIXED = False
S4_INTERLEAVED = False

S1_BASE = DC * 2 * HID + TKV + BDIM  # 1664: kvWT | wbv | uT
S2 = DC * (TKV // 2)  # 1024: kv s-half 0
S3 = DC * (TKV // 2)  # 1024: kv s-half 1
O_WBV = DC * 2 * HID  # 1024 within slab1
O_UT = O_WBV + TKV  # 1536 within slab1
O_QW = S1_BASE  # 1664: qWT within slab1 when QW_IN_S1
O_QT = DC * HID  # 512 within slab4 (legacy layout)


def _sizes():
    if QW_IN_S1:
        return S1_BASE + DC * HID, DC * R + 2  # qWT in s1; s4 = qT | qb | vb
    return S1_BASE, DC * HID + DC * R + 2


def _build():
    S1, S4 = _sizes()
    F32R = mybir.dt.float32r
    nc = bacc.Bacc(None)
    s1 = nc.declare_dram_parameter("s1", [P, S1], F32R, isOutput=False)
    s2 = nc.declare_dram_parameter("s2", [P, S2], F32R, isOutput=False)
    s3 = nc.declare_dram_parameter("s3", [P, S3], F32R, isOutput=False)
    s4 = nc.declare_dram_parameter("s4", [P, S4], F32R, isOutput=False)
    out = nc.declare_dram_parameter("out", [HID, R], F32, isOutput=True)
    scratch = nc.dram_tensor("primer_scratch", [1, R], F32)

    with tile.TileContext(nc) as tc:
        with (
            tc.tile_pool(name="persist", bufs=1) as persist,
            tc.tile_pool(name="psum", bufs=2, space="PSUM") as psum,
            tc.tile_pool(name="psum1", bufs=1, space="PSUM") as psum1,
        ):
            # ---- slab DMAs (sync HWDGE ring, FIFO order = priority) ----
            m1 = persist.tile([P, S1], F32R, tag="m1")
            m2 = persist.tile([P, S2], F32R, tag="m2")
            m3 = persist.tile([P, S3], F32R, tag="m3")
            m4 = persist.tile([P, S4], F32R, tag="m4")
            H = S2 // 2
            _slabs = {
                "1": (m1, s1),
                "1a": (m1[:, :O_WBV], s1[:, :O_WBV]),
                "1b": (m1[:, O_WBV:], s1[:, O_WBV:]),
                "2": (m2, s2),
                "3": (m3, s3),
                "Q0": (m2[:, :H], s2[:, :H]),
                "Q1": (m2[:, H:], s2[:, H:]),
                "Q2": (m3[:, :H], s3[:, :H]),
                "Q3": (m3[:, H:], s3[:, H:]),
                "4": (m4, s4),
                "4a": (m4[:, :O_QT], s4[:, :O_QT]),
                "4b": (m4[:, O_QT:], s4[:, O_QT:]),
            }
            for i_ch, ch in enumerate(SLAB_ORDER.split()):
                mt, st = _slabs[ch]
                eng = nc.scalar if (DUAL_RING and ch in DUAL_RING.split()) else nc.sync
                if S1_SWDGE and i_ch == 0:
                    eng = nc.gpsimd
                eng.dma_start(out=mt[:], in_=st[:])

            # ---- no casts: f32 slabs feed the PE directly as float32r.
            # fp32r matmuls with moving dim >= 256 run at full (bf16) rate;
            # the narrow wbias/qh matmuls pay 2-4x on only 128 cols each.
            kvW = lambda dc: m1[:, dc * 2 * HID : (dc + 1) * 2 * HID]
            wbv = lambda sc: m1[:, O_WBV + sc * P : O_WBV + (sc + 1) * P]
            uTv = m1[:, O_UT : O_UT + R]
            # kv s-chunk sc, d-chunk dc  (sc 0..1 in slab2, 2..3 in slab3)
            if KV_MIXED:
                # slab2: half packing (4KB lines); slab3: quarter packing
                kv = lambda sc, dc: (
                    m2[:, dc * 256 + sc * P : dc * 256 + sc * P + P]
                    if sc < 2
                    else m3[:, (sc - 2) * 512 + dc * P : (sc - 2) * 512 + dc * P + P]
                )
            elif KV_QUARTERS:
                kv = lambda sc, dc: (m2 if sc < 2 else m3)[
                    :, (sc % 2) * 512 + dc * P : (sc % 2) * 512 + dc * P + P
                ]
            else:
                kv = lambda sc, dc: (m2 if sc < 2 else m3)[
                    :, dc * 256 + (sc % 2) * P : dc * 256 + (sc % 2) * P + P
                ]
            if QW_IN_S1:
                qWT = lambda dc: m1[:, O_QW + dc * HID : O_QW + (dc + 1) * HID]
                qTv = lambda dc: m4[:, dc * R : (dc + 1) * R]
            elif S4_INTERLEAVED:
                qWT = lambda dc: m4[:, dc * 256 : dc * 256 + HID]
                qTv = lambda dc: m4[:, dc * 256 + HID : (dc + 1) * 256]
            else:
                qWT = lambda dc: m4[:, dc * HID : (dc + 1) * HID]
                qTv = lambda dc: m4[:, O_QT + dc * R : O_QT + (dc + 1) * R]
            qb = m4[:, S4 - 2 : S4 - 1].bitcast(F32)
            vb = m4[:, S4 - 1 : S4].bitcast(F32)

            # ---- PE warmup: the tensor engine clock ramps 0.6->1.2->2.4GHz
            # with ~3us of sustained work; dummy matmuls during the DMA
            # stream mean the real matmuls run at full clock ----
            warm_sb = persist.tile([P, 256], BF16, tag="warm_sb")
            # engine choice: gpsimd/scalar preambles end ~1us before DVE's,
            # so their memset unblocks the PE warmup earlier
            _ms_eng = {"gps": nc.gpsimd, "dve": nc.vector}[MEMSET_ENG]
            _ms_eng.memset(warm_sb[:], 0.0)
            pwm = psum1.tile([P, 256], F32, tag="pwm")
            for _ in range(N_WARMUP):
                nc.tensor.matmul(pwm[:], lhsT=warm_sb[:, :P], rhs=warm_sb[:])

            def fillers():
                # PE keeps clock ramp through upcoming sem-wait gaps: these
                # have no deps, so they run while the next group's DMA lands.
                for _ in range(INTER_FILLER):
                    nc.tensor.matmul(pwm[:, :P], lhsT=warm_sb[:, :P], rhs=warm_sb[:, :P])

            # ---- expwbT (s,t): lhsT = wbv chunk, rhs = uT ----
            wT_bf = persist.tile([P, SC, R], BF16, tag="wT_bf")
            for sc in range(SC):
                pw = psum.tile([P, R], F32, tag="pw")
                nc.tensor.matmul(pw[:], lhsT=wbv(sc), rhs=uTv)
                nc.scalar.activation(wT_bf[:, sc, :], pw[:], ACT.Exp)

            # ---- qhT (h,t); sigmoid via exp so ACT never switches tables:
            # sigmoid(qh) = 1/(1+e) with e = exp(-(qh + qW_b))  (host sends -qW_b)
            def qh_block():
                pq = psum1.tile([P, R], F32, tag="pq")
                for dc in range(DC):
                    nc.tensor.matmul(
                        pq[:],
                        lhsT=qWT(dc),
                        rhs=qTv(dc),
                        start=(dc == 0),
                        stop=(dc == DC - 1),
                    )
                e_sb = persist.tile([P, R], F32, tag="e_sb")
                nc.scalar.activation(e_sb[:], pq[:], ACT.Exp, bias=qb, scale=-1.0)
                return e_sb

            if QH_EARLY:
                e_sb = qh_block()
                fillers()

            # ---- k/v projections -> ek=exp(k0), ekv=ek*v0  (s,h), with the
            # den/num accumulations (h,t) interleaved per chunk so only the
            # last chunk's matmuls trail the final kv quarter-DMA ----
            ek_bf = persist.tile([P, SC, HID], BF16, tag="ek_bf")
            ekv_bf = persist.tile([P, SC, HID], BF16, tag="ekv_bf")
            pd = psum1.tile([P, R], F32, tag="pd")
            pn = psum1.tile([P, R], F32, tag="pn")
            for sc in range(SC):
                pkv = psum.tile([P, 2 * HID], F32, tag="pkv")
                for dc in range(DC):
                    nc.tensor.matmul(
                        pkv[:],
                        lhsT=kv(sc, dc),
                        rhs=kvW(dc),
                        start=(dc == 0),
                        stop=(dc == DC - 1),
                    )
                nc.scalar.activation(ek_bf[:, sc, :], pkv[:, :HID], ACT.Exp)
                nc.vector.tensor_mul(ekv_bf[:, sc, :], ek_bf[:, sc, :], pkv[:, HID:])
                nc.tensor.matmul(
                    pd[:],
                    lhsT=ek_bf[:, sc, :],
                    rhs=wT_bf[:, sc, :],
                    start=(sc == 0),
                    stop=(sc == SC - 1),
                )
                nc.tensor.matmul(
                    pn[:],
                    lhsT=ekv_bf[:, sc, :],
                    rhs=wT_bf[:, sc, :],
                    start=(sc == 0),
                    stop=(sc == SC - 1),
                )
                if sc < SC - 1:
                    fillers()

            if not QH_EARLY:
                e_sb = qh_block()

            # ---- out = (num + vb*den) / ((1+e)*den) ----
            # vb*den on ACT (Copy with per-partition scale) so no DVE op reads
            # two PSUM tensors at once.  Optionally split into column halves so
            # the first output DMA launches while the second half finishes.
            vbd_sb = persist.tile([P, R], F32, tag="vbd_sb")
            t1_sb = persist.tile([P, R], F32, tag="t1_sb")
            t2_sb = persist.tile([P, R], F32, tag="t2_sb")
            rec_sb = persist.tile([P, R], F32, tag="rec_sb")
            res_sb = persist.tile([P, R], F32, tag="res_sb")
            out_eng = nc.scalar if OUT_SCALAR else nc.sync
            if OUT_PRIMER:
                # tiny DMA gated on a late tensor: rewarms the (cold) output
                # HWDGE ring ~1.5us before the real output store, absorbing
                # its ~1.4us first-descriptor latency
                out_eng.dma_start(out=scratch[:], in_=e_sb[:1, :])
            halves = [slice(0, R // 2), slice(R // 2, R)] if SPLIT_EPI else [slice(0, R)]
            for hs in halves:
                nc.scalar.mul(vbd_sb[:, hs], pd[:, hs], vb)
                nc.vector.scalar_tensor_tensor(
                    t1_sb[:, hs], e_sb[:, hs], 1.0, pd[:, hs],
                    mybir.AluOpType.add, mybir.AluOpType.mult,
                )
                nc.vector.tensor_add(t2_sb[:, hs], vbd_sb[:, hs], pn[:, hs])
                if EPI_DIVIDE:
                    nc.vector.tensor_tensor(
                        res_sb[:, hs], t2_sb[:, hs], t1_sb[:, hs],
                        mybir.AluOpType.divide,
                    )
                else:
                    nc.vector.reciprocal_approx_fast(rec_sb[:, hs], t1_sb[:, hs])
                    nc.vector.tensor_mul(res_sb[:, hs], t2_sb[:, hs], rec_sb[:, hs])
                out_eng.dma_start(out=out[:, hs], in_=res_sb[:, hs])

    nc.finalize()
    return nc


_NC_CACHE = None


def _get_nc():
    global _NC_CACHE
    if _NC_CACHE is None:
        _NC_CACHE = _build()
    return _NC_CACHE


def _make_in_maps(q, kv, qW_w, qW_b, kW_w, kW_b, vW_w, vW_b, w_bias_u, w_bias_v):
    f = lambda a: np.ascontiguousarray(np.asarray(a, dtype=np.float32))
    q, kv = f(q), f(kv)
    kvW = np.concatenate([np.asarray(kW_w), np.asarray(vW_w)], axis=0)  # (2H, DIM)
    # kvWT tiled (P, DC, 2H): [p, dc, n] = kvW[n, dc*P+p]
    kvWT_t = np.transpose(kvW.reshape(2 * HID, DC, P), (2, 1, 0))  # (P, DC, 2H)
    qWT_t = np.transpose(np.asarray(qW_w).reshape(HID, DC, P), (2, 1, 0))  # (P,DC,H)
    wbv = np.asarray(w_bias_v)  # (BDIM, TKV)
    u = np.asarray(w_bias_u)
    qf = q.reshape(B * TQ, DIM)
    if KV_MIXED:
        # slab2 half-style [p, dc, sw(256)], slab3 quarter-style [p, scl, dc, sw]
        halves = [
            np.transpose(kv[b].reshape(2, TKV // 2, DC, P), (3, 0, 2, 1))
            for b in range(B)
        ]
        quarters = [
            np.transpose(kv[b].reshape(2, 2, P, DC, P), (4, 0, 1, 3, 2))
            for b in range(B)
        ]
        kv_s2 = [halves[b][:, 0].reshape(P, -1) for b in range(B)]
        kv_s3 = [quarters[b][:, 1].reshape(P, -1) for b in range(B)]
    elif KV_QUARTERS:
        # [p, sh, sc_local, dc, sw]: kv[b, sh*256 + sc_local*128 + sw, dc*P+p]
        kv_t = [
            np.transpose(kv[b].reshape(2, 2, P, DC, P), (4, 0, 1, 3, 2))
            for b in range(B)
        ]
    else:
        # kv[b] tiled (P, 2, DC, TKV//2): [p, sh, dc, sw] = kv[b, sh*256+sw, dc*P+p]
        kv_t = [
            np.transpose(kv[b].reshape(2, TKV // 2, DC, P), (3, 0, 2, 1))
            for b in range(B)
        ]
    if not KV_MIXED:
        kv_s2 = [kv_t[b][:, 0].reshape(P, -1) for b in range(B)]
        kv_s3 = [kv_t[b][:, 1].reshape(P, -1) for b in range(B)]
    s1_parts = [kvWT_t.reshape(P, -1), wbv, np.zeros((P, R), np.float32)]
    if QW_IN_S1:
        s1_parts.append(qWT_t.reshape(P, -1))
    slab1_shared = np.concatenate(s1_parts, axis=1)
    in_maps = []
    for i in range(N_CORES):
        b = i // (N_CORES // B)
        t0 = (i % (N_CORES // B)) * R
        s1 = slab1_shared.copy()
        s1[:, O_UT : O_UT + R] = u[t0 : t0 + R].T  # (BDIM, R)
        # qT tiled: [p, dc, t] = qf[i*R + t, dc*P+p]
        qT_t = np.transpose(
            qf[i * R : (i + 1) * R].reshape(R, DC, P), (2, 1, 0)
        )  # (P, DC, R)
        nqb = -np.asarray(qW_b, np.float32).reshape(P, 1)
        vbc = np.asarray(vW_b, np.float32).reshape(P, 1)
        if QW_IN_S1:
            s4 = np.concatenate([qT_t.reshape(P, -1), nqb, vbc], axis=1)
        elif S4_INTERLEAVED:
            parts = []
            for dc in range(DC):
                parts += [qWT_t[:, dc, :], qT_t[:, dc, :]]
            s4 = np.concatenate(parts + [nqb, vbc], axis=1)
        else:
            s4 = np.concatenate(
                [qWT_t.reshape(P, -1), qT_t.reshape(P, -1), nqb, vbc], axis=1
            )
        in_maps.append(
            {
                "s1": f(s1),
                "s2": f(kv_s2[b]),
                "s3": f(kv_s3[b]),
                "s4": f(s4),
            }
        )
    return in_maps


def _run(in_maps, trace=False):
    # The shared-pool devices occasionally throw transient
    # NRT_EXEC_UNIT_UNRECOVERABLE errors; the runtime resets the core on the
    # next open, so a short-backoff retry recovers.
    import time

    nc = _get_nc()
    last = None
    for attempt in range(3):
        try:
            return run_bass_kernel_spmd(
                nc, in_maps, core_ids=list(range(N_CORES)), trace=trace
            )
        except Exception as e:  # noqa: BLE001 - retry any runtime failure
            last = e
            time.sleep(2.0 * (attempt + 1))
    raise last


def kernel(**inputs) -> np.ndarray:
    in_maps = _make_in_maps(**inputs)
    res = _run(in_maps)
    out = np.empty((B * TQ, HID), dtype=np.float32)
    for i in range(N_CORES):
        out[i * R : (i + 1) * R] = res.results[i]["out"].T
    return out.reshape(B, TQ, HID)
